# revision 1
# baseline (speedup 1.0000x reference)
"""ExtractTensorPatches kernel for 8 trn2 NeuronCores.

Problem: x (4, 32, 256, 256) f32 -> out (4, 961, 32, 16, 16) f32 with
  out[b, ho*31+wo, c, i, j] = x[b, c, 8*ho+i, 8*wo+j] + EPS * patchsum
  patchsum = sum over the 16x16 patch at (8*ho, 8*wo).

Sharding: pure data parallelism over channels. Core k handles channels
[4k, 4k+4) for all 4 batches.

Design (bf16 end-to-end; gate is rel_err < 2e-2, bf16 keeps ~3e-3):
  partition p = (c, r8): 128 partitions, each owns 8 UNIQUE rows
  (8*r8 .. 8*r8+7) -> loads fully deduplicated (2MB/core bf16).
  Patch row-blocks: half 0 (i<8) of patch ho=r8 and half 1 (i>=8) of
  patch ho=r8-1 both live in partition r8's own rows.
  Per batch b:
    X8  [128, 2048] bf16: one SWDGE load (4KB/partition, 512KB); each
        128-partition SWDGE DMA spreads over 8 SDMA engines (HWDGE
        would cap at the 4-engine set 64-67).
    V8  [128, 256]  bf16: vertical sum of the 8 rows (halving
        tensor_adds, step-1 bf16 -> DVE 2x mode).
    R2  [128, 31]   f32: windowed 16-sum of V8 (DVE reduce).
    PE  : S = selS^T R2, Sp = selSp^T R2 -> PSUM [128, 62] f32; the
        only cross-partition step (selector matrices from host).
    E0/E1 [128, 496] bf16 = EPS * S/Sp broadcast over j, filled by the
        ACT engine reading PSUM.
    OUT [128, 7936] bf16: per half a tensor_tensor add of the
        overlapping-window X8 AP + E (all operands innermost step 1,
        DVE 2x). Stores per half: SWDGE [128, 7936B] ~1MB.
  All 4 loads are prefetched up front; emission is software-pipelined
  (tree/reduce of batch b+1 interleaves with the TT+stores of batch b)
  so the DVE stream never stalls on the PE->ACT latency; the last
  batch's TTs/stores are wo-split so the final store drain is short.
  Host reassembles (ho, i) from (r8, half, il) and upcasts to f32.

Measured: 46.3us (baseline 102.4us): DVE-bound (26us busy) between a
~12us fixed head (engine init + first load) and a ~7us store tail.
"""
import sys

for _p in ("/opt/trn_rl_repo", "/root/.axon_site/_ro/trn_rl_repo"):
    if _p not in sys.path:
        sys.path.append(_p)

import numpy as np

B, C, H, W = 4, 32, 256, 256
WIN, STR = 16, 8
HO = (H - WIN) // STR + 1  # 31
L = HO * HO  # 961
EPS = 1e-6
NCORES = 8
CLOC = C // NCORES  # 4 channels per core
R8 = 32  # row-bands of 8 per channel
HSZ = HO * 8 * WIN  # 3968 elems per half per partition

_nc_cache = {}


def _mk(t, dims, extra_off=0, np_=128):
    """Build a custom AP on a pool tile: partition dim + given free dims."""
    import concourse.bass as bass

    pstep = 1
    for d in t.tensor.shape[1:]:
        pstep *= d
    return bass.AP(
        t.tensor, t.offset + extra_off, [[pstep, np_]] + [list(d) for d in dims]
    )


def build_nc():
    import concourse.bacc as bacc
    import concourse.mybir as mybir
    import concourse.tile as tile
    import concourse.bass as bass

    f32 = mybir.dt.float32
    bf16 = mybir.dt.bfloat16
    nc = bacc.Bacc(
        "TRN2", target_bir_lowering=False, debug=False, num_devices=NCORES
    )
    x = nc.dram_tensor("x", [B, CLOC, H, W], bf16, kind="ExternalInput").ap()
    sel = nc.dram_tensor("sel", [128, 256], f32, kind="ExternalInput").ap()
    out = nc.dram_tensor(
        "out", [B, CLOC, R8, 2, HO, 8, WIN], bf16, kind="ExternalOutput"
    ).ap()

    with tile.TileContext(nc) as tc:
        with (
            tc.tile_pool(name="xin", bufs=4) as xpool,
            tc.tile_pool(name="stats", bufs=3) as spool,
            tc.tile_pool(name="outp", bufs=4) as opool,
            tc.tile_pool(name="selp", bufs=1) as selpool,
            tc.psum_pool(name="ps", bufs=2) as pspool,
        ):
            SEL = selpool.tile([128, 256], f32, tag="SEL")
            nc.sync.dma_start(out=SEL[:, :], in_=sel)

            # ---- phase 1: prefetch all batches' rows.
            Xs = []
            for b in range(B):
                X = xpool.tile([128, 8 * W], bf16, tag="X")
                src = bass.AP(
                    x.tensor,
                    b * CLOC * H * W,
                    [[H * W, CLOC], [STR * W, R8], [1, 8 * W]],
                )
                nc.gpsimd.dma_start(out=_mk(X, [[1, 8 * W]]), in_=src)
                Xs.append(X)

            def emit_front(b):
                """tree + reduce (DVE), patch-sum matmuls (PE), E fills
                (ACT) for batch b; returns what the TT stage needs."""
                X = Xs[b]
                V8 = spool.tile([128, W], bf16, tag="V8")
                T1 = spool.tile([128, 4 * W], bf16, tag="T1")
                nc.vector.tensor_add(
                    _mk(T1, [[1, 4 * W]]),
                    _mk(X, [[1, 4 * W]]),
                    _mk(X, [[1, 4 * W]], extra_off=4 * W),
                )
                T2 = spool.tile([128, 2 * W], bf16, tag="T2")
                nc.vector.tensor_add(
                    _mk(T2, [[1, 2 * W]]),
                    _mk(T1, [[1, 2 * W]]),
                    _mk(T1, [[1, 2 * W]], extra_off=2 * W),
                )
                nc.vector.tensor_add(
                    _mk(V8, [[1, W]]),
                    _mk(T2, [[1, W]]),
                    _mk(T2, [[1, W]], extra_off=W),
                )
                R2 = spool.tile([128, HO], f32, tag="R2")
                nc.vector.reduce_sum(
                    out=_mk(R2, [[1, HO]]),
                    in_=_mk(V8, [[STR, HO], [1, WIN]]),
                    axis=mybir.AxisListType.X,
                )
                # S (cols 0:31) / Sp (cols 31:62) on PE.
                PS = pspool.tile([128, 62], f32, tag="PS")
                nc.tensor.matmul(
                    out=PS[:, 0:HO],
                    lhsT=SEL[:, 0:128],
                    rhs=_mk(R2, [[1, HO]]),
                    start=True,
                    stop=True,
                )
                nc.tensor.matmul(
                    out=PS[:, HO : 2 * HO],
                    lhsT=SEL[:, 128:256],
                    rhs=_mk(R2, [[1, HO]]),
                    start=True,
                    stop=True,
                )
                E0 = spool.tile([128, HO * WIN], bf16, tag="E0")
                E1 = spool.tile([128, HO * WIN], bf16, tag="E1")
                psum_pstep = 1
                for dd in PS.tensor.shape[1:]:
                    psum_pstep *= dd
                for h, Et in ((0, E0), (1, E1)):
                    src_ps = bass.AP(
                        PS.tensor,
                        PS.offset + h * HO,
                        [[psum_pstep, 128], [1, HO], [0, WIN]],
                    )
                    nc.scalar.mul(
                        _mk(Et, [[WIN, HO], [1, WIN]]),
                        src_ps,
                        float(EPS),
                    )
                return (E0, E1)

            def emit_back(b, Es, cuts=((0, HO),)):
                """window-expand TTs (DVE) + stores (SWDGE), optionally
                wo-split to start/finish the store stream earlier."""
                X = Xs[b]
                OUT = opool.tile([128, 2 * HSZ], bf16, tag="OUT")
                for h, Et in ((0, Es[0]), (1, Es[1])):
                    for (w0, w1) in cuts:
                        nwo = w1 - w0
                        nc.vector.tensor_add(
                            _mk(
                                OUT,
                                [[8 * WIN, nwo], [WIN, 8], [1, WIN]],
                                extra_off=h * HSZ + w0 * 8 * WIN,
                            ),
                            _mk(
                                X,
                                [[STR, nwo], [W, 8], [1, WIN]],
                                extra_off=w0 * STR,
                            ),
                            _mk(
                                Et,
                                [[WIN, nwo], [0, 8], [1, WIN]],
                                extra_off=w0 * WIN,
                            ),
                        )
                        dst = bass.AP(
                            out.tensor,
                            b * CLOC * R8 * 2 * HSZ + h * HSZ + w0 * 8 * WIN,
                            [[2 * HSZ, 128], [1, nwo * 8 * WIN]],
                        )
                        nc.gpsimd.dma_start(
                            out=dst,
                            in_=OUT[
                                :,
                                h * HSZ
                                + w0 * 8 * WIN : h * HSZ
                                + w1 * 8 * WIN,
                            ],
                        )

            # ---- phase 2: software-pipelined emission. DVE stream is
            # tree0 red0 tree1 red1 TT0 tree2 red2 TT1 ... so the DVE
            # never waits on the PE->ACT latency of the same batch.
            CUT3 = ((0, 16), (16, HO))
            Es_prev = emit_front(0)
            for b in range(1, B):
                Es = emit_front(b)
                emit_back(b - 1, Es_prev)
                Es_prev = Es
            emit_back(B - 1, Es_prev, cuts=CUT3)

    nc.compile()
    return nc


def get_nc():
    if "nc" not in _nc_cache:
        _nc_cache["nc"] = build_nc()
    return _nc_cache["nc"]


def _sel_matrix() -> np.ndarray:
    selS = np.eye(128, dtype=np.float32) + np.eye(128, k=-1, dtype=np.float32)
    selSp = np.eye(128, dtype=np.float32) + np.eye(128, k=1, dtype=np.float32)
    return np.ascontiguousarray(
        np.concatenate([selS, selSp], axis=1)
    )  # [128, 256]


def make_in_maps(x: np.ndarray):
    import ml_dtypes

    xb = np.asarray(x, dtype=np.float32).astype(ml_dtypes.bfloat16)
    sel = _sel_matrix()
    return [
        {
            "x": np.ascontiguousarray(xb[:, k * CLOC : (k + 1) * CLOC]),
            "sel": sel,
        }
        for k in range(NCORES)
    ]


def kernel(x: np.ndarray) -> np.ndarray:
    from concourse.bass_utils import run_bass_kernel_spmd

    nc = get_nc()
    res = run_bass_kernel_spmd(nc, make_in_maps(x), list(range(NCORES)))
    # res[k]["out"]: (B, CLOC, r8=32, half=2, wo, il=8, j).
    # Patch rows i<8 live at (r8=ho, half0); i>=8 at (r8=ho+1, half1).
    arr = np.stack([np.asarray(r["out"]) for r in res.results], axis=0)
    own = arr[:, :, :, 0:31, 0]  # (k, B, CLOC, ho, wo, 8, 16)
    prv = arr[:, :, :, 1:32, 1]
    comb = np.concatenate([own, prv], axis=5)  # il dim -> 16
    return np.ascontiguousarray(
        comb.transpose(1, 3, 4, 0, 2, 5, 6)
        .reshape(B, L, C, WIN, WIN)
        .astype(np.float32)
    )

